# revision 29
# baseline (speedup 1.0000x reference)
"""Trainium2 Bass kernel for CrossAttentionFusion (v3).

Reference computation (per batch b):
    Q = q_w @ f1 + q_b          (O, N)   f1 = features1[b] as (C, N)
    K = k_w @ f2 + k_b          (O, N)
    V = v_w @ f2 + v_b          -> used as (N, O)
    A = softmax(Q^T K / sqrt(O))  over keys          (N, N)
    att = A @ V                  (N, O)
    Z = o_w @ att^T + o_b        (O, N)
    out = GroupNorm(8 groups over O, spatial N)(Z) * gn_w + gn_b

Sharding: pure data-parallel, batch b -> NeuronCore b (B=8, 8 cores).

Key structural points:
 * Score reassociation: softmax is invariant to per-query shifts, so
       S'[k,q] = f2[:,k]^T G[:,q] + r_k,   G = (q_w^T k_w)^T f1,
       r_k = (k_w^T q_b)^T f2[:,k] * scale
   The K and Q projections disappear (raw f2 tiles are the stationary
   operands of the score matmul); G costs one projection instead of
   two; r_k rides along as a 257th output column of the V projection
   and enters exp() through the activation bias port. The k_b and
   q_b.k_b score terms are per-query constants -> dropped.
 * DMA: three rings (sync/scalar/gpsimd), few big descriptors, issued
   in exactly PE consumption order. G0/G1 + first f2 cols arrive ~2us
   in; chunk-0 scores start ~4us; V tiles and remaining G chunks are
   interleaved into the chunk-0 score stream.
 * Denominator: pairwise bf16 tree -> ones-vector matmul -> DVE
   reciprocal of the [1,jw] row -> gpsimd partition_broadcast (no PE /
   scalar work).
 * GroupNorm rstd = exp(-0.5*ln(var+eps)): Ln and Exp share one
   activation table, so no ACT_TABLE_LOAD on the critical path.
 * Z and the output are bf16 (half the DMA); host converts to f32.
"""

import numpy as np

B = 8
C = 256
O = 256
N = 2304
NKT = 18  # key tiles of 128
VW = O + 1  # V projection width: O cols of V + 1 col of r_k*scale
GROUPS = 8
EPS = 1e-5
SCALE = float(O) ** -0.5

_BUILD_CACHE = {}


def _subs(jw):
    return [(s, min(512, jw - s)) for s in range(0, jw, 512)]


def _build_nc():
    import concourse.mybir as mybir
    import concourse.tile as tile
    from concourse import bacc
    from concourse.bass import ts

    F32 = mybir.dt.float32
    F32R = mybir.dt.float32r
    BF16 = mybir.dt.bfloat16
    AF = mybir.ActivationFunctionType
    ALU = mybir.AluOpType

    nc = bacc.Bacc("TRN2", target_bir_lowering=False)

    f1_d = nc.dram_tensor("features1", [C, N], BF16, kind="ExternalInput")
    f2_d = nc.dram_tensor("features2", [C, N], BF16, kind="ExternalInput")
    # host-precomputed: HT = q_w.T @ k_w (lhsT for G), vwT_aug = [v_w.T | k_w.T q_b * scale]
    ht_d = nc.dram_tensor("HT", [C, O], BF16, kind="ExternalInput")
    vwT_d = nc.dram_tensor("vwT_aug", [C, VW], BF16, kind="ExternalInput")
    owT_d = nc.dram_tensor("owT", [O, O], F32R, kind="ExternalInput")
    vbb_d = nc.dram_tensor("vb_bcast", [128, VW], F32, kind="ExternalInput")
    ob_d = nc.dram_tensor("o_b", [O], F32, kind="ExternalInput")
    gnw_d = nc.dram_tensor("gn_w", [O], F32, kind="ExternalInput")
    gnb_d = nc.dram_tensor("gn_b", [O], F32, kind="ExternalInput")
    gsel_d = nc.dram_tensor("gsel", [128, 2 * GROUPS], F32, kind="ExternalInput")
    gselT_d = nc.dram_tensor("gselT", [GROUPS, 2 * 128], F32, kind="ExternalInput")
    onesb_d = nc.dram_tensor("ones_bf", [128, 1], BF16, kind="ExternalInput")
    magic_d = nc.dram_tensor("magic", [GROUPS, 1], mybir.dt.int32, kind="ExternalInput")
    out_d = nc.dram_tensor("out", [O, N], BF16, kind="ExternalOutput")

    with tile.TileContext(nc) as tc:
        with (
            tc.tile_pool(name="consts", bufs=1) as consts,
            tc.tile_pool(name="weights", bufs=1) as wpool,
            tc.tile_pool(name="acts", bufs=1) as apool,
            tc.tile_pool(name="feat", bufs=1) as fpool,
            tc.tile_pool(name="ppool", bufs=2) as ppool,
            tc.tile_pool(name="tpool", bufs=1) as tpool,
            tc.tile_pool(name="sbm", bufs=2) as sbm,
        ):
            # ---- persistent tiles ----
            ht = [wpool.tile([128, O], BF16, name=f"ht{t}") for t in range(2)]
            vwT = [wpool.tile([128, VW], BF16, name=f"vwT{t}") for t in range(2)]
            owT = [wpool.tile([128, O], F32R, name=f"owT{t}") for t in range(2)]
            vb_bc = consts.tile([128, VW], F32, name="vb_bc")
            ones_bf = consts.tile([128, 1], BF16, name="ones_bf")
            gsel = consts.tile([128, 2 * GROUPS], F32, name="gsel")
            gselT = consts.tile([GROUPS, 2 * 128], F32, name="gselT")
            ob_c = [consts.tile([128, 1], F32, name=f"ob{t}") for t in range(2)]
            gnw_c = [consts.tile([128, 1], F32, name=f"gnw{t}") for t in range(2)]
            gnb_c = [consts.tile([128, 1], F32, name=f"gnb{t}") for t in range(2)]
            magic_c = consts.tile([GROUPS, 1], mybir.dt.int32, name="magic_c")

            f1sb = [fpool.tile([128, N], BF16, name=f"f1sb{t}") for t in range(2)]
            f2sb = [apool.tile([128, N], BF16, name=f"f2sb{t}") for t in range(2)]
            G = [apool.tile([128, N], BF16, name=f"G{t}") for t in range(2)]
            V = [apool.tile([128, VW], BF16, name=f"V{k}") for k in range(NKT)]
            Z = [apool.tile([128, N], BF16, name=f"Z{t}") for t in range(2)]
            st_sums = [apool.tile([128, 2], F32, name=f"st{t}") for t in range(2)]
            for t in range(2):
                nc.vector.memset(st_sums[t], 0.0)

            # ---- DMA issue: 3 rings, consumption order; G0 deps (f1 cols
            # 0:512 of both halves + HT) land first, one per ring ----
            nc.sync.dma_start(out=f1sb[0][:, 0:512], in_=f1_d[ts(0, 128), 0:512])
            nc.sync.dma_start(out=f1sb[1][:, 512:1024], in_=f1_d[ts(1, 128), 512:1024])
            nc.sync.dma_start(out=f2sb[0][:, 0:1152], in_=f2_d[ts(0, 128), 0:1152])
            nc.sync.dma_start(out=f2sb[0][:, 1152:N], in_=f2_d[ts(0, 128), 1152:N])
            nc.scalar.dma_start(out=f1sb[1][:, 0:512], in_=f1_d[ts(1, 128), 0:512])
            nc.scalar.dma_start(out=f1sb[0][:, 512:1024], in_=f1_d[ts(0, 128), 512:1024])
            nc.scalar.dma_start(out=f2sb[1][:, 0:1152], in_=f2_d[ts(1, 128), 0:1152])
            nc.scalar.dma_start(out=f2sb[1][:, 1152:N], in_=f2_d[ts(1, 128), 1152:N])
            # gpsimd: HT, vwT+vb (V proj), f1 tails, owT, late consts
            for t in range(2):
                nc.gpsimd.dma_start(out=ht[t], in_=ht_d[ts(t, 128), :])
            for t in range(2):
                nc.gpsimd.dma_start(out=vwT[t], in_=vwT_d[ts(t, 128), :])
            nc.gpsimd.dma_start(out=vb_bc, in_=vbb_d[:, :])
            nc.gpsimd.dma_start(out=f1sb[0][:, 1024:N], in_=f1_d[ts(0, 128), 1024:N])
            nc.gpsimd.dma_start(out=f1sb[1][:, 1024:N], in_=f1_d[ts(1, 128), 1024:N])
            for t in range(2):
                nc.gpsimd.dma_start(out=owT[t], in_=owT_d[ts(t, 128), :])
            nc.gpsimd.dma_start(out=ones_bf, in_=onesb_d[:, :])
            nc.gpsimd.dma_start(out=gsel, in_=gsel_d[:, :])
            nc.gpsimd.dma_start(out=gselT, in_=gselT_d[:, :])
            nc.gpsimd.dma_start(out=magic_c, in_=magic_d[:, :])
            for lst, src in ((ob_c, ob_d), (gnw_c, gnw_d), (gnb_c, gnb_d)):
                for t in range(2):
                    nc.gpsimd.dma_start(out=lst[t], in_=src[ts(t, 128)].unsqueeze(1))

            GCH = [(0, 512), (512, 512), (1024, 512), (1536, 512), (2048, 256)]

            with tc.tile_pool(name="sps", bufs=2, space="PSUM") as sps:

                def scores_nk(j0, jw, nk):
                    sp = sps.tile([128, 1024], F32, tag="sp", name="sp")
                    for s0, sw in _subs(jw):
                        nc.tensor.matmul(
                            sp[:, s0 : s0 + sw],
                            f2sb[0][:, ts(nk, 128)],
                            G[0][:, j0 + s0 : j0 + s0 + sw],
                            start=True,
                            stop=False,
                        )
                        nc.tensor.matmul(
                            sp[:, s0 : s0 + sw],
                            f2sb[1][:, ts(nk, 128)],
                            G[1][:, j0 + s0 : j0 + s0 + sw],
                            start=False,
                            stop=True,
                        )
                    pt = ppool.tile([128, 1024], BF16, tag=f"p{nk}", name=f"pt{nk}")
                    nc.scalar.activation(
                        pt[:, :jw],
                        sp[:, :jw],
                        AF.Exp,
                        bias=V[nk][:, O : O + 1],
                        scale=SCALE,
                    )
                    return pt

                def tree(P, jw, pref=""):
                    tr = [
                        tpool.tile(
                            [128, jw], BF16, tag=f"tr{pref}{i}", name=f"tr{pref}{i}"
                        )
                        for i in range(9)
                    ]
                    for i in range(9):
                        nc.vector.tensor_add(
                            tr[i][:, :jw], P[2 * i][:, :jw], P[2 * i + 1][:, :jw]
                        )
                    for i in range(4):
                        nc.vector.tensor_add(
                            tr[2 * i][:, :jw], tr[2 * i][:, :jw], tr[2 * i + 1][:, :jw]
                        )
                    nc.vector.tensor_add(tr[0][:, :jw], tr[0][:, :jw], tr[2][:, :jw])
                    nc.vector.tensor_add(tr[4][:, :jw], tr[4][:, :jw], tr[6][:, :jw])
                    nc.vector.tensor_add(tr[0][:, :jw], tr[0][:, :jw], tr[4][:, :jw])
                    nc.vector.tensor_add(tr[0][:, :jw], tr[0][:, :jw], tr[8][:, :jw])
                    return tr[0]

                # ---- phase A fused with chunk-0 scores ----
                P0 = []
                with (
                    tc.tile_pool(name="vps", bufs=2, space="PSUM") as vps,
                    tc.tile_pool(name="gpsA", bufs=2, space="PSUM") as gpsA,
                ):

                    def g_chunk(ci):
                        c0, cw = GCH[ci]
                        csl = slice(c0, c0 + cw)
                        for t in range(2):
                            gp = gpsA.tile([128, 512], F32, tag="gp", name="gp")
                            nc.tensor.matmul(
                                gp[:, :cw], ht[0][:, ts(t, 128)], f1sb[0][:, csl],
                                start=True, stop=False,
                            )
                            nc.tensor.matmul(
                                gp[:, :cw], ht[1][:, ts(t, 128)], f1sb[1][:, csl],
                                start=False, stop=True,
                            )
                            nc.scalar.copy(G[t][:, csl], gp[:, :cw])

                    def v_tile(nk):
                        vp = vps.tile([128, VW], F32, tag="vp", name="vp")
                        nc.tensor.matmul(
                            vp, f2sb[0][:, ts(nk, 128)], vwT[0], start=True, stop=False
                        )
                        nc.tensor.matmul(
                            vp, f2sb[1][:, ts(nk, 128)], vwT[1], start=False, stop=True
                        )
                        nc.vector.tensor_add(V[nk], vp, vb_bc)

                    g_chunk(0)
                    g_chunk(1)
                    for nk in range(NKT):
                        v_tile(nk)
                        P0.append(scores_nk(0, 1024, nk))
                        if nk == 8:
                            g_chunk(2)
                        elif nk == 11:
                            g_chunk(3)
                        elif nk == 14:
                            g_chunk(4)
                tr0_c0 = tree(P0, 1024)

                # ---- rest of phase B ----
                with (
                    tc.tile_pool(name="ops", bufs=2, space="PSUM") as ops,
                    tc.tile_pool(name="zps", bufs=1, space="PSUM") as zps,
                    tc.tile_pool(name="dps", bufs=1, space="PSUM") as dps,
                ):

                    def denom(tr0, s0, sw):
                        ssl = slice(s0, s0 + sw)
                        dn = dps.tile([1, 512], F32, tag="d", name="dn")
                        nc.tensor.matmul(
                            dn[:, :sw], ones_bf, tr0[:, ssl], start=True, stop=True
                        )
                        rrow = sbm.tile([1, 512], F32, tag="rrow", name="rrow")
                        nc.vector.reciprocal_approx_fast(rrow[:, :sw], dn[:, :sw])
                        bcs = sbm.tile([128, 512], F32, tag="bcs", name="bcs")
                        nc.gpsimd.partition_broadcast(bcs[:, :sw], rrow[:, :sw])
                        return bcs

                    def attn_o(P, s0, sw, o):
                        ssl = slice(s0, s0 + sw)
                        op = ops.tile([128, 512], F32, tag="op", name="op")
                        for nk in range(NKT):
                            nc.tensor.matmul(
                                op[:, :sw],
                                V[nk][:, ts(o, 128)],
                                P[nk][:, ssl],
                                start=(nk == 0),
                                stop=(nk == NKT - 1),
                            )
                        return op

                    def comp_fin(j0, oacc, bcs, s0, sw, split=False):
                        ATs = []
                        for o in range(2):
                            at = sbm.tile(
                                [128, 512], F32R, tag=f"at{o}", name=f"at{o}"
                            )
                            nc.vector.tensor_mul(
                                at[:, :sw], oacc[o][:, :sw], bcs[:, :sw]
                            )
                            ATs.append(at)
                        # output projection sub-chunk: Z[p, sw]
                        zsl = slice(j0 + s0, j0 + s0 + sw)
                        for p in range(2):
                            zp = zps.tile([128, 512], F32, tag="zp", name="zp")
                            nc.tensor.matmul(
                                zp[:, :sw], owT[0][:, ts(p, 128)], ATs[0][:, :sw],
                                start=True, stop=False,
                            )
                            nc.tensor.matmul(
                                zp[:, :sw], owT[1][:, ts(p, 128)], ATs[1][:, :sw],
                                start=False, stop=True,
                            )
                            # halve the evac/square on the final subchunk so
                            # the DVE/scalar chain pipelines before GN stats
                            nh = 2 if split else 1
                            hw_ = sw // nh
                            for h in range(nh):
                                hz = slice(j0 + s0 + h * hw_, j0 + s0 + (h + 1) * hw_)
                                hp = slice(h * hw_, (h + 1) * hw_)
                                part = sbm.tile(
                                    [128, 2], F32, tag=f"part{p}", name=f"part{p}"
                                )
                                nc.vector.tensor_scalar(
                                    Z[p][:, hz],
                                    zp[:, hp],
                                    ob_c[p],
                                    0.0,
                                    op0=ALU.add,
                                    op1=ALU.add,
                                    accum_out=part[:, 0:1],
                                )
                                sqs = sbm.tile([128, 512], BF16, tag="sqs", name="sqs")
                                nc.scalar.activation(
                                    sqs[:, :hw_],
                                    Z[p][:, hz],
                                    AF.Square,
                                    accum_out=part[:, 1:2],
                                )
                                nc.vector.tensor_add(st_sums[p], st_sums[p], part)

                    # Explicit schedule: the small chunk's scores interleave
                    # into chunk-0's compute (the PE work gives its exps
                    # slack), and the small chunk's compute interleaves into
                    # chunk-2's scores (ditto for chunk-2's exps).
                    P1, P2 = [], []

                    def sscore(a, b):
                        for nk in range(a, b):
                            P1.append(scores_nk(2048, 256, nk))

                    op00 = attn_o(P0, 0, 512, 0)
                    sscore(0, 4)
                    bcs00 = denom(tr0_c0, 0, 512)
                    op01 = attn_o(P0, 0, 512, 1)
                    sscore(4, 8)
                    comp_fin(0, [op00, op01], bcs00, 0, 512)
                    op10 = attn_o(P0, 512, 512, 0)
                    sscore(8, 12)
                    bcs01 = denom(tr0_c0, 512, 512)
                    op11 = attn_o(P0, 512, 512, 1)
                    sscore(12, 18)
                    comp_fin(0, [op10, op11], bcs01, 512, 512)
                    tr0_s = tree(P1, 256, pref="s")

                    for nk in range(0, 6):
                        P2.append(scores_nk(1024, 1024, nk))
                    sop0 = attn_o(P1, 0, 256, 0)
                    for nk in range(6, 12):
                        P2.append(scores_nk(1024, 1024, nk))
                    sbcs = denom(tr0_s, 0, 256)
                    sop1 = attn_o(P1, 0, 256, 1)
                    for nk in range(12, NKT):
                        P2.append(scores_nk(1024, 1024, nk))
                    comp_fin(2048, [sop0, sop1], sbcs, 0, 256)
                    tr0_c2 = tree(P2, 1024)

                    for s0, sw in _subs(1024):
                        o0 = attn_o(P2, s0, sw, 0)
                        bcs = denom(tr0_c2, s0, sw)
                        o1 = attn_o(P2, s0, sw, 1)
                        comp_fin(1024, [o0, o1], bcs, s0, sw, split=(s0 == 512))

            # ---- phase C: GroupNorm finalization ----
            with (
                tc.tile_pool(name="gns", bufs=2) as gns,
                tc.tile_pool(name="gout", bufs=1) as gout,
                tc.tile_pool(name="gps", bufs=2, space="PSUM") as gps,
            ):
                gst = gps.tile([GROUPS, 2], F32, tag="gst", name="gst")
                nc.tensor.matmul(
                    gst, gsel[:, 0:GROUPS], st_sums[0], start=True, stop=False
                )
                nc.tensor.matmul(
                    gst,
                    gsel[:, GROUPS : 2 * GROUPS],
                    st_sums[1],
                    start=False,
                    stop=True,
                )
                # per-group [mean, rstd] on 8 partitions
                ms = gns.tile([GROUPS, 2], F32, tag="ms", name="ms")
                inv_cnt = 1.0 / (32.0 * N)
                nc.vector.tensor_scalar_mul(ms, gst, inv_cnt)  # [mean, E[x^2]]
                m2 = gns.tile([GROUPS, 1], F32, tag="m2", name="m2")
                nc.vector.tensor_mul(m2, ms[:, 0:1], ms[:, 0:1])  # mean^2
                ve = gns.tile([GROUPS, 1], F32, tag="ve", name="ve")
                nc.vector.scalar_tensor_tensor(
                    ve, ms[:, 1:2], EPS, m2, op0=ALU.add, op1=ALU.subtract
                )  # var+eps
                # rstd = rsqrt(var+eps) on DVE only: bit-trick seed + 2 Newton
                # steps (no scalar engine, no activation-table load)
                I32 = mybir.dt.int32
                sh = gns.tile([GROUPS, 1], I32, tag="sh", name="sh")
                nc.vector.tensor_scalar(
                    sh, ve.bitcast(I32), 1, 0, op0=ALU.arith_shift_right, op1=ALU.bypass
                )
                y0i = gns.tile([GROUPS, 1], I32, tag="y0i", name="y0i")
                nc.vector.tensor_sub(y0i, magic_c, sh)
                y = y0i.bitcast(F32)
                # one Newton step (seed err ~3.4% -> ~0.2%; plenty for GN)
                yy = gns.tile([GROUPS, 1], F32, tag="yy", name="yy")
                nc.vector.tensor_mul(yy, y, y)
                nc.vector.tensor_mul(yy, yy, ve)
                nc.vector.tensor_scalar(yy, yy, -0.5, 1.5, op0=ALU.mult, op1=ALU.add)
                nc.vector.tensor_mul(ms[:, 1:2], y, yy)
                for p in range(2):
                    pst = gps.tile([128, 2], F32, tag="pst", name="pst")
                    nc.tensor.matmul(
                        pst, gselT[:, ts(p, 128)], ms, start=True, stop=True
                    )
                    a_col = gns.tile([128, 1], F32, tag="a_col", name="a_col")
                    nc.vector.tensor_mul(a_col, pst[:, 1:2], gnw_c[p])
                    t_col = gns.tile([128, 1], F32, tag="t_col", name="t_col")
                    nc.vector.scalar_tensor_tensor(
                        t_col, pst[:, 0:1], a_col, gnb_c[p],
                        op0=ALU.mult, op1=ALU.subtract,
                    )  # mean*a - gnb
                    # out = Z*a - t ; half-chunks, one live tile each
                    for hi, h0 in enumerate((0, 1152)):
                        outp = gout.tile(
                            [128, 1152], BF16, tag=f"outp{p}{hi}", name="outp"
                        )
                        nc.vector.tensor_scalar(
                            outp,
                            Z[p][:, h0 : h0 + 1152],
                            a_col,
                            t_col,
                            op0=ALU.mult,
                            op1=ALU.subtract,
                        )
                        eng = (nc.sync, nc.gpsimd, nc.scalar, nc.sync)[2 * p + hi]
                        eng.dma_start(out=out_d[ts(p, 128), h0 : h0 + 1152], in_=outp)

    nc.finalize()
    return nc


def _get_nc():
    if "nc" not in _BUILD_CACHE:
        _BUILD_CACHE["nc"] = _build_nc()
    return _BUILD_CACHE["nc"]


def _make_in_maps(inputs):
    import ml_dtypes

    f1 = np.ascontiguousarray(
        np.asarray(inputs["features1"], dtype=np.float32)
        .reshape(B, C, N)
        .astype(ml_dtypes.bfloat16)
    )
    f2 = np.ascontiguousarray(
        np.asarray(inputs["features2"], dtype=np.float32)
        .reshape(B, C, N)
        .astype(ml_dtypes.bfloat16)
    )

    def g(k):
        return np.asarray(inputs[k], dtype=np.float32)

    gsel = np.zeros((128, 2 * GROUPS), np.float32)
    gselT = np.zeros((GROUPS, 2 * 128), np.float32)
    for t in range(2):
        for gl in range(4):
            grp = 4 * t + gl
            gsel[gl * 32 : (gl + 1) * 32, GROUPS * t + grp] = 1.0
            gselT[grp, 128 * t + gl * 32 : 128 * t + (gl + 1) * 32] = 1.0

    qw, kw, vw = g("q_w"), g("k_w"), g("v_w")
    HT = np.ascontiguousarray((qw.T @ kw).astype(ml_dtypes.bfloat16))
    rcol = (kw.T @ g("q_b")) * SCALE  # [C]
    vwT_aug = np.concatenate([vw.T, rcol[:, None]], axis=1)
    vb_bcast = np.zeros((128, VW), np.float32)
    vb_bcast[:, :O] = g("v_b")[None, :]
    shared = {
        "HT": HT,
        "vwT_aug": np.ascontiguousarray(vwT_aug.astype(ml_dtypes.bfloat16)),
        "owT": np.ascontiguousarray(g("o_w").T),
        "vb_bcast": vb_bcast,
        "o_b": g("o_b"),
        "gn_w": g("gn_w"),
        "gn_b": g("gn_b"),
        "gsel": gsel,
        "gselT": gselT,
        "ones_bf": np.ones((128, 1), ml_dtypes.bfloat16),
        "magic": np.full((GROUPS, 1), 0x5F3759DF, np.int32),
    }
    return [{"features1": f1[i], "features2": f2[i], **shared} for i in range(B)]


def run(inputs, trace=False):
    from concourse.bass_utils import run_bass_kernel_spmd

    nc = _get_nc()
    in_maps = _make_in_maps(inputs)
    res = run_bass_kernel_spmd(nc, in_maps, core_ids=list(range(B)), trace=trace)
    out = np.stack(
        [np.asarray(res.results[i]["out"]).astype(np.float32) for i in range(B)]
    )
    return out.reshape(B, O, 48, 48), res


def kernel(**inputs):
    out, _ = run(inputs, trace=False)
    return out
